# revision 12
# baseline (speedup 1.0000x reference)
"""Distributed 3-layer GATv2 kernel for one TRN2 chip (8 NeuronCores).

Contract: kernel(**inputs) takes the FULL inputs from setup_inputs()
(x [50000,128] f32, edge_index [2,800000] int, params list) and returns
the FULL output [50000, 64] f32.

Sharding: nodes are split into 8 contiguous chunks (graph partitioned by
destination node); edges (incl. self-loops) are sorted by dst so each
core owns the edges targeting its chunk. Per layer, each core computes
xl/xr for its node chunk, all-gathers xl (fp16) so every core holds the
full source-feature table, then processes its edges in "super-batches"
(SB) of <=127 dst nodes and T=TLO+THI tiles of 128 edge slots:

  - dma_gather (int16 indices, 4 SWDGE queues) pulls xl[src] rows; the
    table is larger than the int16 range, so edges are statically split
    into a lo region (first TLO tiles, table rows < 32768) and a hi
    region (THI tiles, gathered from the sliced table base +32768)
  - a third dma_gather pulls xr[dst] per edge slot; m = xl[src]+xr[dst]
    via one DVE add  [128 x T*HC] fp16
  - e = sum_c att * leaky_relu(m) per edge/head (DVE)
  - unnormalized weights exp(e); one-hot matrices (edge-slot -> SB node
    slot, built on-chip via is_equal) give segment sums on TensorE:
    denom[n,h] and scat[n,:] = sum_e exp(e)*m
  - epilogue per SB: out = scat/denom - xr[n] + bias (uses sum alpha=1),
    ELU for layers 1-2, pad rows masked to 0

Node features live in a SB-major padded layout ([NSB*128] rows per core)
so every DMA has static addressing; the host builds all index tables and
unpacks the final SB-major output. Layer 3 (single head, 64 ch) runs
with channels padded to 128 so gather rows stay 256B-aligned.
"""

import os
import sys
import numpy as np

_REPO = "/opt/trn_rl_repo"
if _REPO not in sys.path:
    sys.path.insert(0, _REPO)

bass = bacc = tile = mybir = bass_rust = None
IndirectOffsetOnAxis = run_bass_kernel_spmd = None
F16 = F32 = I32 = I16 = None

NEG_SLOPE = 0.2
SPLIT = 32768          # int16 index ceiling for dma_gather


def _lazy_import():
    """Import the bass stack on first use.

    Importing concourse alongside jax-on-axon in one process can wedge
    plain jax device transfers, so the import is deferred until inputs
    have been converted to numpy.
    """
    global bass, bacc, tile, mybir, bass_rust
    global IndirectOffsetOnAxis, run_bass_kernel_spmd, F16, F32, I32, I16
    if bass is not None:
        return
    import bass_rust as _br
    import concourse.bass as _bass
    import concourse.bacc as _bacc
    import concourse.tile as _tile
    from concourse import mybir as _mybir
    from concourse.bass import IndirectOffsetOnAxis as _ioa
    from concourse.bass_utils import run_bass_kernel_spmd as _run
    bass, bacc, tile, mybir, bass_rust = _bass, _bacc, _tile, _mybir, _br
    IndirectOffsetOnAxis, run_bass_kernel_spmd = _ioa, _run
    F16 = _mybir.dt.float16
    F32 = _mybir.dt.float32
    I32 = _mybir.dt.int32
    I16 = _mybir.dt.int16


def _ap_with(ap, pattern):
    """Rebuild an AP with an explicit [[step, count], ...] pattern."""
    return bass_rust.AP(ap.tensor, ap.offset, [list(p) for p in pattern])


def _rep_mid(ap2d, t):
    """[P, F] -> [P, t, F] with the middle dim broadcast (step 0)."""
    a = list(ap2d.ap)
    assert len(a) == 2
    return _ap_with(ap2d, [list(a[0]), [0, t], list(a[1])])


# ---------------------------------------------------------------------------
# configuration


class Cfg:
    def __init__(self, n_nodes=50000, n_cores=8, tlo=10, thi=7,
                 layers=None, out_cols=64, split=SPLIT, gmax=1024):
        self.n_nodes = n_nodes
        self.n_cores = n_cores
        assert n_nodes % n_cores == 0
        self.chunk = n_nodes // n_cores
        self.tlo = tlo
        self.thi = thi
        self.T = tlo + thi
        self.sb_nodes_max = 127         # pad ldst = 127 never real
        self.split = split
        self.out_cols = out_cols
        self.gmax = gmax
        # (din, H, C, elu); layer-3 channels padded 64 -> 128
        self.layers = layers or [(128, 4, 64, True), (256, 4, 64, True),
                                 (256, 1, 128, False)]


# ---------------------------------------------------------------------------
# host-side graph preprocessing (shared across layers)


def _wrap16(vals, num):
    """Index list -> [128, num//16] int16 wrapped layout (i -> [i%16, i//16])."""
    a = np.zeros((16, num // 16), np.int16)
    v = np.asarray(vals, np.int64)
    i = np.arange(len(v))
    a[i % 16, i // 16] = v.astype(np.int16)
    return np.tile(a, (8, 1))


def preprocess(cfg, edge_index):
    """Returns per-core index tables in SB-major layout + global NSB."""
    n = cfg.n_nodes
    tlo_cap = cfg.tlo * 128
    thi_cap = cfg.thi * 128
    src = np.concatenate([edge_index[0], np.arange(n, dtype=np.int64)])
    dst = np.concatenate([edge_index[1], np.arange(n, dtype=np.int64)])
    order = np.argsort(dst, kind="stable")
    src, dst = src[order].astype(np.int64), dst[order].astype(np.int64)

    # First pass: node -> SB packing needs grow values, which need NSB.
    # grow depends on NSB only through per-core row padding; iterate:
    # start from a lower bound, recompute until stable (converges fast).
    nsb = int(np.ceil(cfg.chunk / cfg.sb_nodes_max))
    for _ in range(12):
        R = nsb * 128
        # global SB-major row requires the packing itself; to break the
        # cycle, pack using a provisional grow based on this nsb.
        cores, ok = _pack(cfg, src, dst, nsb)
        if ok:
            break
        nsb += 1
    else:
        raise RuntimeError("SB packing did not converge")

    R = nsb * 128
    sbrow = np.full(n, -1, np.int64)
    for k, c in enumerate(cores):
        for i, (nn0, nn, _, _) in enumerate(c["sbs"]):
            if nn:
                nodes = c["node_lo"] + nn0 + np.arange(nn)
                sbrow[nodes] = i * 128 + np.arange(nn)
    assert (sbrow >= 0).all()
    core_of = np.arange(n) // cfg.chunk
    grow = core_of * R + sbrow

    for k, c in enumerate(cores):
        T = cfg.T
        ilo = np.zeros((nsb, 128, tlo_cap // 16), np.int16)
        ihi = np.zeros((nsb, 128, thi_cap // 16), np.int16)
        ixr = np.zeros((nsb, 128, T * 128 // 16), np.int16)
        ldst = np.full((nsb, 128, T), 127, np.float16)
        mask = np.zeros((nsb, 128, 1), np.float32)
        real = np.zeros((nsb, 128), bool)
        for i, (nn0, nn, e0, ne) in enumerate(c["sbs"]):
            if ne:
                s = c["src"][e0:e0 + ne]
                d = c["dst"][e0:e0 + ne]
                g = grow[s]
                lo_m = g < cfg.split
                ldv = (sbrow[d] - i * 128)
                # lo region: slots (i%128, i//16...) of tiles [0, tlo)
                glo, dlo, llo = g[lo_m], sbrow[d][lo_m], ldv[lo_m]
                ghi, dhi, lhi = g[~lo_m] - cfg.split, sbrow[d][~lo_m], ldv[~lo_m]
                assert len(glo) <= tlo_cap and len(ghi) <= thi_cap
                lo_v = np.zeros(tlo_cap, np.int64)
                lo_v[:len(glo)] = glo
                ilo[i] = _wrap16(lo_v, tlo_cap)
                hi_v = np.zeros(thi_cap, np.int64)
                hi_v[:len(ghi)] = ghi
                ihi[i] = _wrap16(hi_v, thi_cap)
                xr_v = np.zeros(T * 128, np.int64)
                xr_v[:len(dlo)] = dlo
                xr_v[tlo_cap:tlo_cap + len(dhi)] = dhi
                ixr[i] = _wrap16(xr_v, T * 128)
                sl = np.arange(len(llo))
                ldst[i, sl % 128, sl // 128] = llo.astype(np.float16)
                sl = np.arange(len(lhi))
                ldst[i, sl % 128, cfg.tlo + sl // 128] = lhi.astype(np.float16)
            mask[i, :nn, 0] = 1.0
            real[i, :nn] = True
        c.update(ilo=ilo, ihi=ihi, ixr=ixr, ldst=ldst, mask=mask, real=real)

    return cores, nsb, sbrow


def _pack(cfg, src, dst, nsb):
    """Pack nodes into SBs given a provisional nsb (for grow values)."""
    n = cfg.n_nodes
    R = nsb * 128
    tlo_cap = cfg.tlo * 128
    thi_cap = cfg.thi * 128

    # provisional sbrow assuming this nsb (computed per core as we pack)
    cores = []
    # grow for split decisions: need sbrow of SRC nodes, i.e. the packing
    # of OTHER cores too. Two-phase: pack by edge counts only first is
    # wrong; instead compute split on the fly using a provisional grow
    # from an equal-rows layout, then verify after real packing.
    # Simpler robust approach: decide lo/hi from a provisional grow based
    # on node id (nodes laid out in order, row = sb*128 + offset), which
    # equals the final layout exactly because packing is in node order.
    # We just don't know each core's sb boundaries until packed, so pack
    # all cores first by capacity on *estimated* lo/hi membership, then
    # recheck with the exact grow; if any SB overflows, retry via caller.
    est_sbrow = np.full(n, -1, np.int64)
    for k in range(cfg.n_cores):
        lo_n, hi_n = k * cfg.chunk, (k + 1) * cfg.chunk
        lo_e = np.searchsorted(dst, lo_n)
        hi_e = np.searchsorted(dst, hi_n)
        s_k, d_k = src[lo_e:hi_e], dst[lo_e:hi_e]
        cores.append(dict(src=s_k, dst=d_k, node_lo=lo_n))

    # iterate packing until the lo/hi split assignment is stable
    grow_est = (np.arange(n) // cfg.chunk) * R + \
        np.minimum(np.arange(n) % cfg.chunk, R - 1)  # rough initial guess
    for _ in range(4):
        sbrow_new = np.full(n, -1, np.int64)
        ok = True
        for k, c in enumerate(cores):
            d_loc = c["dst"] - c["node_lo"]
            deg = np.bincount(d_loc, minlength=cfg.chunk)
            is_lo_e = grow_est[c["src"]] < cfg.split
            lo_deg = np.bincount(d_loc[is_lo_e], minlength=cfg.chunk)
            sbs = []
            n0 = e0 = 0
            while n0 < cfg.chunk:
                nn = nlo = nhi = 0
                while n0 + nn < cfg.chunk and nn < cfg.sb_nodes_max:
                    a_lo = lo_deg[n0 + nn]
                    a_hi = deg[n0 + nn] - a_lo
                    if nlo + a_lo > tlo_cap or nhi + a_hi > thi_cap:
                        break
                    nlo += a_lo
                    nhi += a_hi
                    nn += 1
                if nn == 0:
                    ok = False
                    break
                ne = int(deg[n0:n0 + nn].sum())
                sbs.append((n0, nn, e0, ne))
                nodes = c["node_lo"] + n0 + np.arange(nn)
                sbrow_new[nodes] = len(sbs) * 128 - 128 + np.arange(nn)
                n0 += nn
                e0 += ne
            if not ok or len(sbs) > nsb:
                ok = False
                break
            while len(sbs) < nsb:
                sbs.append((cfg.chunk, 0, len(c["src"]), 0))
            c["sbs"] = sbs
        if not ok:
            return cores, False
        grow_new = (np.arange(n) // cfg.chunk) * R + sbrow_new
        if (grow_new == grow_est).all():
            return cores, True
        grow_est = grow_new
    return cores, (grow_new == grow_est).all()


# ---------------------------------------------------------------------------
# program builder


def build_program(cfg, nsb):
    _lazy_import()
    T = cfg.T
    TLO, THI = cfg.tlo, cfg.thi
    R = nsb * 128
    W = cfg.n_cores
    nc = bacc.Bacc("TRN2", target_bir_lowering=False, debug=True)

    din0 = cfg.layers[0][0]
    x0 = nc.dram_tensor("x0", [R, din0], F16, kind="ExternalInput")
    ilo_t = nc.dram_tensor("ilo", [nsb, 128, TLO * 8], I16, kind="ExternalInput")
    ihi_t = nc.dram_tensor("ihi", [nsb, 128, THI * 8], I16, kind="ExternalInput")
    ixr_t = nc.dram_tensor("ixr", [nsb, 128, T * 8], I16, kind="ExternalInput")
    ldst_t = nc.dram_tensor("ldst", [nsb, 128, T], F16, kind="ExternalInput")
    mask_t = nc.dram_tensor("mask", [nsb, 128, 1], F32, kind="ExternalInput")
    iota_t = nc.dram_tensor("iota", [128, 128], F16, kind="ExternalInput")

    lt = []
    for l, (din, H, C, elu) in enumerate(cfg.layers):
        HC = H * C
        lt.append(dict(
            wl=nc.dram_tensor(f"wl{l}", [din, HC], F16, kind="ExternalInput"),
            wr=nc.dram_tensor(f"wr{l}", [din, HC], F16, kind="ExternalInput"),
            attb=nc.dram_tensor(f"attb{l}", [128, HC], F16, kind="ExternalInput"),
            biasb=nc.dram_tensor(f"biasb{l}", [128, HC], F32, kind="ExternalInput"),
            xl_loc=nc.dram_tensor(f"xl_loc{l}", [R, HC], F16),
            xr_loc=nc.dram_tensor(f"xr_loc{l}", [R, HC], F16),
            xl_full=nc.dram_tensor(f"xl_full{l}", [W * R, HC], F16,
                                   addr_space="Shared"),
        ))
        if l < len(cfg.layers) - 1:
            lt[l]["x_next"] = nc.dram_tensor(f"x{l + 1}", [R, HC], F16)

    out_t = nc.dram_tensor("out", [R, cfg.out_cols], F32, kind="ExternalOutput")

    replica_groups = [list(range(W))]

    with tile.TileContext(nc) as tc:
        with (
            tc.tile_pool(name="const", bufs=1) as constp,
            tc.tile_pool(name="wts", bufs=1) as wtsp,
            tc.tile_pool(name="node", bufs=3) as nodep,
            tc.tile_pool(name="npsum", bufs=2, space="PSUM") as npsum,
            tc.tile_pool(name="edge", bufs=2) as edgep,
            tc.tile_pool(name="escr", bufs=2) as escrp,
            tc.tile_pool(name="epsum", bufs=2, space="PSUM") as epsum,
        ):
            iota_s = constp.tile([128, 128], F16)
            nc.sync.dma_start(out=iota_s[:], in_=iota_t[:])

            x_prev = x0
            for l, (din, H, C, elu) in enumerate(cfg.layers):
                HC = H * C
                KC = din // 128
                t_ = lt[l]

                # ---- weights + per-layer constants to SBUF
                wl_s = wtsp.tile([128, KC * HC], F16, name=f"wl_s{l}")
                wr_s = wtsp.tile([128, KC * HC], F16, name=f"wr_s{l}")
                attb_s = wtsp.tile([128, HC], F16, name=f"attb_s{l}")
                biasb_s = wtsp.tile([128, HC], F32, name=f"biasb_s{l}")
                for kc in range(KC):
                    nc.sync.dma_start(out=wl_s[:, kc * HC:(kc + 1) * HC],
                                      in_=t_["wl"][kc * 128:(kc + 1) * 128, :])
                    nc.sync.dma_start(out=wr_s[:, kc * HC:(kc + 1) * HC],
                                      in_=t_["wr"][kc * 128:(kc + 1) * 128, :])
                nc.sync.dma_start(out=attb_s[:], in_=t_["attb"][:])
                nc.sync.dma_start(out=biasb_s[:], in_=t_["biasb"][:])

                # ---- node phase: xl_loc = x @ Wl, xr_loc = x @ Wr (fp16)
                for t in range(nsb):
                    r0 = t * 128
                    xT = nodep.tile([128, KC * 128], F16, name="xT")
                    for kc in range(KC):
                        nc.sync.dma_start(
                            out=xT[:, kc * 128:(kc + 1) * 128],
                            in_=x_prev[r0:r0 + 128, kc * 128:(kc + 1) * 128],
                            transpose=True)
                    ps_l = npsum.tile([128, HC], F32, name="ps_l")
                    ps_r = npsum.tile([128, HC], F32, name="ps_r")
                    for kc in range(KC):
                        nc.tensor.matmul(
                            out=ps_l[:], lhsT=xT[:, kc * 128:(kc + 1) * 128],
                            rhs=wl_s[:, kc * HC:(kc + 1) * HC],
                            start=(kc == 0), stop=(kc == KC - 1))
                    for kc in range(KC):
                        nc.tensor.matmul(
                            out=ps_r[:], lhsT=xT[:, kc * 128:(kc + 1) * 128],
                            rhs=wr_s[:, kc * HC:(kc + 1) * HC],
                            start=(kc == 0), stop=(kc == KC - 1))
                    xl16 = nodep.tile([128, HC], F16, name="xl16")
                    xr16 = nodep.tile([128, HC], F16, name="xr16")
                    nc.scalar.copy(xl16[:], ps_l[:])
                    nc.vector.tensor_copy(xr16[:], ps_r[:])
                    nc.sync.dma_start(out=t_["xl_loc"][r0:r0 + 128, :],
                                      in_=xl16[:])
                    nc.sync.dma_start(out=t_["xr_loc"][r0:r0 + 128, :],
                                      in_=xr16[:])

                # ---- all-gather the xl table
                nc.gpsimd.collective_compute(
                    "AllGather", mybir.AluOpType.bypass,
                    ins=[t_["xl_loc"][:]],
                    outs=[t_["xl_full"][:]],
                    replica_groups=replica_groups,
                )

                # ---- edge phase
                for sb in range(nsb):
                    ilo_s = edgep.tile([128, TLO * 8], I16, name="ilo_s")
                    ihi_s = edgep.tile([128, THI * 8], I16, name="ihi_s")
                    ixr_s = edgep.tile([128, T * 8], I16, name="ixr_s")
                    ldst_s = edgep.tile([128, T], F16, name="ldst_s")
                    mask_s = edgep.tile([128, 1], F32, name="mask_s")
                    nc.sync.dma_start(out=ilo_s[:], in_=ilo_t[sb])
                    nc.sync.dma_start(out=ihi_s[:], in_=ihi_t[sb])
                    nc.sync.dma_start(out=ixr_s[:], in_=ixr_t[sb])
                    nc.sync.dma_start(out=ldst_s[:], in_=ldst_t[sb])
                    nc.sync.dma_start(out=mask_s[:], in_=mask_t[sb])

                    # m = xl[src] (lo | hi) + xr[dst]
                    M = edgep.tile([128, T * HC], F16, name="M")
                    Xr = edgep.tile([128, T * HC], F16, name="Xr")
                    def gather(dst_ap2d, table, idx_s, n_idx, gmax=cfg.gmax):
                        # chunk a gather into <=gmax-index dma_gather calls;
                        # both the out AP and the wrapped idx tile slice
                        # contiguously (idx i lives at [i%16, i//16])
                        for c0 in range(0, n_idx, gmax):
                            cn = min(gmax, n_idx - c0)
                            nc.gpsimd.dma_gather(
                                out_ap=dst_ap2d[
                                    :, (c0 // 128) * HC:
                                    ((c0 + cn) // 128) * HC].rearrange(
                                        "p (n d) -> p n d", d=HC),
                                in_ap=table,
                                idxs_ap=idx_s[:, c0 // 16:(c0 + cn) // 16],
                                num_idxs=cn, num_idxs_reg=cn,
                                elem_size=HC, queue_num=0)

                    gather(M[:, :TLO * HC], t_["xl_full"][:cfg.split, :],
                           ilo_s, TLO * 128)
                    gather(M[:, TLO * HC:], t_["xl_full"][cfg.split:, :],
                           ihi_s, THI * 128)
                    gather(Xr[:], t_["xr_loc"][:], ixr_s, T * 128)
                    nc.vector.tensor_tensor(out=M[:], in0=M[:], in1=Xr[:],
                                            op=mybir.AluOpType.add)

                    # e = sum_c att * lrelu(m) : [128, T*H] f32
                    L = edgep.tile([128, T * HC], F16, name="L")
                    nc.vector.tensor_scalar_mul(L[:], M[:], NEG_SLOPE)
                    nc.vector.tensor_tensor(out=L[:], in0=L[:], in1=M[:],
                                            op=mybir.AluOpType.max)
                    P = edgep.tile([128, T * HC], F16, name="P")
                    nc.vector.tensor_tensor(
                        out=P[:].rearrange("p (t hc) -> p t hc", hc=HC),
                        in0=L[:].rearrange("p (t hc) -> p t hc", hc=HC),
                        in1=_rep_mid(attb_s[:], T), op=mybir.AluOpType.mult)
                    e_f = escrp.tile([128, T * H], F32, name="e_f")
                    nc.vector.tensor_reduce(
                        out=e_f[:],
                        in_=P[:].rearrange("p (th c) -> p th c", c=C),
                        axis=mybir.AxisListType.X, op=mybir.AluOpType.add)
                    expe = escrp.tile([128, T * H], F16, name="expe")
                    nc.scalar.activation(expe[:], e_f[:],
                                         mybir.ActivationFunctionType.Exp)

                    # one-hot [edge-slot -> node-slot] for all T tiles
                    O_all = edgep.tile([128, T * 128], F16, name="O_all")
                    nc.vector.tensor_tensor(
                        out=O_all[:].rearrange("p (t n) -> p t n", n=128),
                        in0=ldst_s[:].to_broadcast([128, T, 128]),
                        in1=_rep_mid(iota_s[:], T),
                        op=mybir.AluOpType.is_equal)

                    # w = m * expe (broadcast expe over C)
                    Wt = edgep.tile([128, T * HC], F16, name="Wt")
                    nc.vector.tensor_tensor(
                        out=Wt[:].rearrange("p (th c) -> p th c", c=C),
                        in0=M[:].rearrange("p (th c) -> p th c", c=C),
                        in1=expe[:].to_broadcast([128, T * H, C]),
                        op=mybir.AluOpType.mult)

                    # segment sums on PE: denom [128, H], scat [128, HC]
                    den_ps = epsum.tile([128, H], F32, name="den_ps")
                    scat_ps = epsum.tile([128, HC], F32, name="scat_ps")
                    for j in range(T):
                        Oj = O_all[:, j * 128:(j + 1) * 128]
                        nc.tensor.matmul(
                            out=den_ps[:], lhsT=Oj,
                            rhs=expe[:, j * H:(j + 1) * H],
                            start=(j == 0), stop=(j == T - 1))
                        nc.tensor.matmul(
                            out=scat_ps[:], lhsT=Oj,
                            rhs=Wt[:, j * HC:(j + 1) * HC],
                            start=(j == 0), stop=(j == T - 1))

                    # epilogue: out = scat/denom - xr + bias  (then ELU)
                    rden = escrp.tile([128, H], F32, name="rden")
                    nc.vector.tensor_scalar_max(rden[:], den_ps[:], 1e-6)
                    nc.vector.reciprocal(rden[:], rden[:])
                    xr_sb = escrp.tile([128, HC], F16, name="xr_sb")
                    nc.sync.dma_start(
                        out=xr_sb[:],
                        in_=t_["xr_loc"][sb * 128:(sb + 1) * 128, :])
                    o = escrp.tile([128, HC], F32, name="o")
                    nc.vector.tensor_tensor(
                        out=o[:].rearrange("p (h c) -> p h c", c=C),
                        in0=scat_ps[:].rearrange("p (h c) -> p h c", c=C),
                        in1=rden[:].to_broadcast([128, H, C]),
                        op=mybir.AluOpType.mult)
                    nc.vector.tensor_tensor(out=o[:], in0=o[:], in1=xr_sb[:],
                                            op=mybir.AluOpType.subtract)
                    nc.vector.tensor_tensor(out=o[:], in0=o[:], in1=biasb_s[:],
                                            op=mybir.AluOpType.add)
                    if elu:
                        neg = escrp.tile([128, HC], F32, name="neg")
                        nc.vector.tensor_scalar_min(neg[:], o[:], 0.0)
                        en = escrp.tile([128, HC], F32, name="en")
                        nc.scalar.activation(en[:], neg[:],
                                             mybir.ActivationFunctionType.Exp)
                        nc.vector.tensor_scalar_max(o[:], o[:], 0.0)
                        nc.vector.tensor_tensor(out=o[:], in0=o[:], in1=en[:],
                                                op=mybir.AluOpType.add)
                        x16 = escrp.tile([128, HC], F16, name="x16")
                        # (o - 1) * mask : fold into one tensor_scalar
                        nc.vector.tensor_scalar(
                            out=x16[:], in0=o[:], scalar1=-1.0,
                            scalar2=mask_s[:, :1], op0=mybir.AluOpType.add,
                            op1=mybir.AluOpType.mult)
                        nc.sync.dma_start(
                            out=t_["x_next"][sb * 128:(sb + 1) * 128, :],
                            in_=x16[:])
                    else:
                        of = escrp.tile([128, HC], F32, name="of")
                        nc.vector.tensor_scalar_mul(of[:], o[:],
                                                    mask_s[:, :1])
                        nc.sync.dma_start(
                            out=out_t[sb * 128:(sb + 1) * 128, :],
                            in_=of[:, :cfg.out_cols])

                if l < len(cfg.layers) - 1:
                    x_prev = t_["x_next"]

    nc.compile()
    return nc


# ---------------------------------------------------------------------------
# host wrapper


def make_in_maps(cfg, cores, nsb, x, params):
    """Build per-core input dicts."""
    R = nsb * 128
    iota = np.broadcast_to(np.arange(128, dtype=np.float16)[None, :],
                           (128, 128)).copy()
    shared = {"iota": iota}
    for l, (din, H, C, elu) in enumerate(cfg.layers):
        HC = H * C
        p = params[l]
        wl = np.asarray(p["Wl"], np.float32)
        wr = np.asarray(p["Wr"], np.float32)
        att = np.asarray(p["att"], np.float32)
        bias = np.asarray(p["bias"], np.float32)
        hc_real = att.shape[0] * att.shape[1]
        if hc_real < HC:          # pad channels (layer 3: 64 -> 128)
            wl = np.pad(wl, ((0, 0), (0, HC - hc_real)))
            wr = np.pad(wr, ((0, 0), (0, HC - hc_real)))
            att = np.pad(att, ((0, 0), (0, C - att.shape[1])))
            bias = np.pad(bias, (0, HC - hc_real))
        shared[f"wl{l}"] = wl.astype(np.float16)
        shared[f"wr{l}"] = wr.astype(np.float16)
        shared[f"attb{l}"] = np.broadcast_to(
            att.reshape(1, HC), (128, HC)).astype(np.float16).copy()
        shared[f"biasb{l}"] = np.broadcast_to(
            bias.reshape(1, HC), (128, HC)).astype(np.float32).copy()

    in_maps = []
    for k, c in enumerate(cores):
        x0 = np.zeros((R, cfg.layers[0][0]), np.float16)
        for i, (nn0, nn, _, _) in enumerate(c["sbs"]):
            if nn:
                rows = c["node_lo"] + nn0 + np.arange(nn)
                x0[i * 128:i * 128 + nn] = x[rows].astype(np.float16)
        m = dict(shared)
        m.update(x0=x0, ilo=c["ilo"], ihi=c["ihi"], ixr=c["ixr"],
                 ldst=c["ldst"], mask=c["mask"])
        in_maps.append(m)
    return in_maps


def unshard_output(cfg, cores, nsb, results):
    out = np.zeros((cfg.n_nodes, cfg.out_cols), np.float32)
    for k, c in enumerate(cores):
        o = results[k]["out"]
        real = c["real"].reshape(-1)
        out[k * cfg.chunk:(k + 1) * cfg.chunk] = \
            o.reshape(nsb * 128, cfg.out_cols)[real]
    return out


_PROGRAM_CACHE = {}


def kernel(x, edge_index, params):
    x = np.asarray(x).astype(np.float32)
    edge_index = np.asarray(edge_index)
    params = [{k: np.asarray(v) for k, v in p.items()} for p in params]
    _lazy_import()
    cfg = Cfg()
    cores, nsb, _ = preprocess(cfg, edge_index.astype(np.int64))
    key = (cfg.n_nodes, cfg.n_cores, cfg.T, nsb)
    if key not in _PROGRAM_CACHE:
        _PROGRAM_CACHE[key] = build_program(cfg, nsb)
    nc = _PROGRAM_CACHE[key]
    in_maps = make_in_maps(cfg, cores, nsb, x, params)
    res = run_bass_kernel_spmd(nc, in_maps, core_ids=list(range(cfg.n_cores)))
    return unshard_output(cfg, cores, nsb, res.results)
